# revision 8
# baseline (speedup 1.0000x reference)
"""Trainium2 Bass kernel: weighted-KDE avoid-distance (retrieval_knn).

dist[n] = mean_m exp(-0.5 * sum_d (means[m,d]-samples[n,d])^2 / stds[m,d])
out     = -dist + max(dist) + min(dist)

Strategy: data-parallel over the N=8192 samples axis across 8 NeuronCores
(1024 samples each; every core holds the full means/stds buffer).

Per-core math is reformulated as one K=194 matmul + fused exp-accumulate:
  logp[n,m] = sB.mB + s2.w' + a[m]
    w'  = -0.5/std,  sB = -2*s,  mB = m*w',  s2 = s*s,  a[m] = sum_d m^2*w'
All operands are split hi/lo in bf16 (hi = bf16(x), lo = bf16(x - hi)) so the
TensorE bf16 matmul reproduces fp32-level accuracy (~2^-18 per operand):
  pass1 (K=128): [sB_hi sB_lo s2_hi s2_lo] x [mB_hi mB_hi w'_hi w'_hi]
  pass2 (K=128): [sB_hi s2_hi ones(64)]    x [mB_lo w'_lo  mq_hi mq_lo]
with mq = m^2*w' (the a[m] term, summed by the matmul itself via ones-rows).
ScalarE then does exp with a fused per-partition accumulate over the free
(m) axis; bias = -ln(2048) folds the mean's 1/M into the exponent.

The final flip (-dist + max + min) is a trivial O(N) op done on host after
gathering the 8 shards.
"""

import sys

import numpy as np

for _p in ("/opt/trn_rl_repo",):
    if _p not in sys.path:
        sys.path.insert(0, _p)

N, M, D = 8192, 2048, 32
N_CORES = 8
NSH = N // N_CORES        # 1024 samples per core
MT = M // 128             # 16 mean tiles
CT = NSH // 128           # 8 sample chunks per core
MJ = M // 512             # 4 matmul moving slices
LN_M = float(np.log(M))   # ln(2048); exp bias folds the 1/M mean

_CACHE = {}


def _build_nc(reps: int = 1):
    # reps>1 repeats the whole compute body inside one NEFF (used only by
    # test.py to measure per-iteration HW time by wall-clock delta).
    import concourse.bacc as bacc
    import concourse.tile as tile
    from concourse import mybir

    f32 = mybir.dt.float32
    bf16 = mybir.dt.bfloat16
    AF = mybir.ActivationFunctionType
    OP = mybir.AluOpType
    AX = mybir.AxisListType

    nc = bacc.Bacc("TRN2", target_bir_lowering=False, debug=False)

    samples_d = nc.dram_tensor("samples", [NSH, D], f32, kind="ExternalInput")
    means_d = nc.dram_tensor("means", [M, D], f32, kind="ExternalInput")
    stds_d = nc.dram_tensor("stds", [M, D], f32, kind="ExternalInput")
    dist_d = nc.dram_tensor("dist", [NSH], f32, kind="ExternalOutput")

    with tile.TileContext(nc) as tc:
        with (
            tc.tile_pool(name="persist", bufs=1) as pp,
            tc.tile_pool(name="psum", bufs=2, space="PSUM") as psp,
            tc.tile_pool(name="expo", bufs=4) as xp,
            tc.tile_pool(name="accp", bufs=2) as ap_,
        ):
          for _rep in range(reps):
            # ---- load inputs (m / n on partitions, (tile, d) on free) ----
            means_nat = pp.tile([128, MT, D], f32)
            stds_nat = pp.tile([128, MT, D], f32)
            samp_nat = pp.tile([128, CT, D], f32)
            nc.sync.dma_start(means_nat[:], means_d.ap().rearrange("(t p) d -> p t d", p=128))
            nc.sync.dma_start(stds_nat[:], stds_d.ap().rearrange("(t p) d -> p t d", p=128))
            nc.sync.dma_start(samp_nat[:], samples_d.ap().rearrange("(t p) d -> p t d", p=128))

            # ---- mean-side features, natural layout ----
            r = pp.tile([128, MT, D], f32)       # 1/std
            mB = pp.tile([128, MT, D], f32)      # m * w' = -0.5*m/std
            t2 = pp.tile([128, MT, D], f32)      # m^2 * w'
            nc.vector.reciprocal(r[:], stds_nat[:])
            nc.vector.scalar_tensor_tensor(
                mB[:], means_nat[:], -0.5, r[:], op0=OP.mult, op1=OP.mult)
            nc.vector.tensor_mul(t2[:], means_nat[:], mB[:])

            # packed tiles: per m-tile 128 bf16 feature columns, to be DMA-transposed
            packed1 = pp.tile([128, MT, 128], bf16)
            packed2 = pp.tile([128, MT, 128], bf16)
            # pass1 mean rows: [mB_hi, mB_hi, w'_hi, w'_hi]
            nc.scalar.copy(packed1[:, :, 0:D], mB[:])                       # mB_hi (cast)
            nc.scalar.copy(packed1[:, :, D:2 * D], packed1[:, :, 0:D])      # dup
            nc.scalar.mul(packed1[:, :, 2 * D:3 * D], r[:], -0.5)           # w'_hi (cast)
            nc.scalar.copy(packed1[:, :, 3 * D:4 * D], packed1[:, :, 2 * D:3 * D])
            # pass2 mean rows: [mB_lo, w'_lo, mq_hi, mq_lo]
            nc.vector.scalar_tensor_tensor(                                  # mB - mB_hi
                packed2[:, :, 0:D], mB[:], 1.0, packed1[:, :, 0:D],
                op0=OP.mult, op1=OP.subtract)
            nc.vector.scalar_tensor_tensor(                                  # -0.5*r - w'_hi
                packed2[:, :, D:2 * D], r[:], -0.5, packed1[:, :, 2 * D:3 * D],
                op0=OP.mult, op1=OP.subtract)
            nc.scalar.copy(packed2[:, :, 2 * D:3 * D], t2[:])                # mq_hi (cast)
            nc.vector.scalar_tensor_tensor(                                  # mq - mq_hi
                packed2[:, :, 3 * D:4 * D], t2[:], 1.0, packed2[:, :, 2 * D:3 * D],
                op0=OP.mult, op1=OP.subtract)

            # ---- sample-side features ----
            s2 = pp.tile([128, CT, D], f32)
            nc.vector.tensor_mul(s2[:], samp_nat[:], samp_nat[:])
            spacked = pp.tile([128, CT, 128], bf16)  # [sB_hi, sB_lo, s2_hi, s2_lo]
            nc.scalar.mul(spacked[:, :, 0:D], samp_nat[:], -2.0)             # sB_hi
            nc.vector.scalar_tensor_tensor(
                spacked[:, :, D:2 * D], samp_nat[:], -2.0, spacked[:, :, 0:D],
                op0=OP.mult, op1=OP.subtract)                                # sB_lo
            nc.scalar.copy(spacked[:, :, 2 * D:3 * D], s2[:])                # s2_hi
            nc.vector.scalar_tensor_tensor(
                spacked[:, :, 3 * D:4 * D], s2[:], 1.0, spacked[:, :, 2 * D:3 * D],
                op0=OP.mult, op1=OP.subtract)                                # s2_lo

            # ---- transposes (feature-major for the matmul), bf16 DMA xbar ----
            rhs1 = pp.tile([128, M], bf16)
            rhs2 = pp.tile([128, M], bf16)
            s1T = pp.tile([128, NSH], bf16)
            for t in range(MT):
                nc.sync.dma_start(rhs1[:, t * 128:(t + 1) * 128], packed1[:, t, :], transpose=True)
                nc.sync.dma_start(rhs2[:, t * 128:(t + 1) * 128], packed2[:, t, :], transpose=True)
            for c in range(CT):
                nc.sync.dma_start(s1T[:, c * 128:(c + 1) * 128], spacked[:, c, :], transpose=True)
            # pass2 sample rows: [sB_hi(0:32), s2_hi(32:64), ones(64:128)] = K 128
            # (ones rows pair with the mq_hi/mq_lo rows of rhs2 to sum a[m])
            s2T = pp.tile([128, NSH], bf16)
            nc.vector.tensor_copy(s2T[0:D, :], s1T[0:D, :])
            nc.vector.tensor_copy(s2T[D:2 * D, :], s1T[2 * D:3 * D, :])
            nc.vector.memset(s2T[2 * D:4 * D, :], 1.0)

            # ---- main loop: matmul pairs + exp-accumulate ----
            ebias = pp.tile([128, 1], f32)   # exp bias: -ln(M) folds the 1/M mean
            nc.vector.memset(ebias[:], -LN_M)
            dist_sb = pp.tile([128, CT], f32)
            for c in range(CT):
                ps = psp.tile([128, M], f32)  # 4 PSUM banks
                lhs1 = s1T[:, c * 128:(c + 1) * 128]
                lhs2 = s2T[:, c * 128:(c + 1) * 128]
                for j in range(MJ):
                    sl = slice(j * 512, (j + 1) * 512)
                    nc.tensor.matmul(ps[:, sl], lhsT=lhs1, rhs=rhs1[:, sl],
                                     start=True, stop=False)
                    nc.tensor.matmul(ps[:, sl], lhsT=lhs2, rhs=rhs2[:, sl],
                                     start=False, stop=True)
                acc = ap_.tile([128, MJ], f32)
                for j in range(MJ):
                    eo = xp.tile([128, 512], bf16)
                    nc.scalar.activation(eo[:], ps[:, j * 512:(j + 1) * 512],
                                         AF.Exp, bias=ebias[:], scale=1.0,
                                         accum_out=acc[:, j:j + 1])
                nc.vector.reduce_sum(dist_sb[:, c:c + 1], acc[:], axis=AX.X)

            nc.sync.dma_start(dist_d.ap().rearrange("(c p) -> p c", p=128), dist_sb[:])

    nc.compile()
    return nc


def _get_nc():
    if "nc" not in _CACHE:
        _CACHE["nc"] = _build_nc()
    return _CACHE["nc"]


def kernel(samples: np.ndarray, means: np.ndarray, stds: np.ndarray) -> np.ndarray:
    from concourse.bass_utils import run_bass_kernel_spmd

    samples = np.ascontiguousarray(samples, dtype=np.float32)
    means = np.ascontiguousarray(means, dtype=np.float32)
    stds = np.ascontiguousarray(stds, dtype=np.float32)

    nc = _get_nc()
    in_maps = [
        {"samples": samples[i * NSH:(i + 1) * NSH], "means": means, "stds": stds}
        for i in range(N_CORES)
    ]
    res = run_bass_kernel_spmd(nc, in_maps, list(range(N_CORES)))
    dist = np.concatenate([res.results[i]["dist"] for i in range(N_CORES)])
    return (-dist + dist.max() + dist.min()).astype(np.float32)


# revision 33
# speedup vs baseline: 509.7448x; 509.7448x over previous
"""Trainium2 Bass kernel: weighted-KDE avoid-distance (retrieval_knn).

dist[n] = mean_m exp(-0.5 * sum_d (means[m,d]-samples[n,d])^2 / stds[m,d])
out     = -dist + max(dist) + min(dist)

Strategy: data-parallel over the N=8192 samples axis across 8 NeuronCores
(1024 samples each; every core holds the full means/stds buffer).

Per-core math is reformulated as one K=256 matmul + fused exp-accumulate:
  logp[n,m] = sB.mB + s2.w' + a[m]
    w'  = -0.5/std,  sB = -2*s,  mB = m*w',  s2 = s*s,  a[m] = sum_d m^2*w'
All operands are split hi/lo in bf16 (hi = bf16(x), lo = bf16(x - hi)) so the
TensorE bf16 matmul reproduces fp32-level accuracy (~2^-18 per operand):
  pass1 (K=128): [sB_hi sB_lo s2_hi s2_lo] x [mB_hi mB_hi w'_hi w'_hi]
  pass2 (K=128): [sB_hi s2_hi ones(64)]    x [mB_lo w'_lo  mq_hi mq_lo]
with mq = m^2*w' (the a[m] term, summed by the matmul itself via ones-rows).
ScalarE does one exp per 128-sample chunk over the whole [128, 2048] PSUM
tile with a fused per-partition accumulate over the free (m) axis; bias
-ln(2048) folds the mean's 1/M into the exponent.

Feature-major operands are produced by computing features in natural layout
and packing them as bf16 columns; the mean side bounces through DRAM and
DMA-transposes back (2-byte xbar transpose, pipelined in halves), while the
sample side transposes on the TensorE against a bf16 identity.

The final flip (-dist + max + min) is a trivial O(N) op done on host after
gathering the 8 shards.
"""

import sys

import numpy as np

for _p in ("/opt/trn_rl_repo", "/root/.axon_site/_ro/trn_rl_repo"):
    if _p not in sys.path:
        sys.path.insert(0, _p)

N, M, D = 8192, 2048, 32
N_CORES = 8
NSH = N // N_CORES        # 1024 samples per core
MT = M // 128             # 16 mean tiles
CT = NSH // 128           # 8 sample chunks per core
MJ = M // 512             # 4 matmul moving slices
LN_M = float(np.log(M))   # ln(2048); exp bias folds the 1/M mean

_CACHE = {}


def _build_nc(reps: int = 1):
    # reps>1 repeats the whole compute body inside one NEFF (used only by
    # test.py to measure per-iteration HW time by wall-clock delta).
    import concourse.bacc as bacc
    import concourse.tile as tile
    from concourse import mybir
    from concourse.masks import make_identity

    f32 = mybir.dt.float32
    bf16 = mybir.dt.bfloat16
    AF = mybir.ActivationFunctionType
    OP = mybir.AluOpType

    nc = bacc.Bacc("TRN2", target_bir_lowering=False, debug=False)

    samples_d = nc.dram_tensor("samples", [NSH, D], f32, kind="ExternalInput")
    means_d = nc.dram_tensor("means", [M, D], f32, kind="ExternalInput")
    stds_d = nc.dram_tensor("stds", [M, D], f32, kind="ExternalInput")
    dist_d = nc.dram_tensor("dist", [NSH], f32, kind="ExternalOutput")
    # DRAM bounce buffers for the 2-byte xbar transposes (mean side)
    stg1_d = nc.dram_tensor("stg1", [M, 128], bf16)
    stg2_d = nc.dram_tensor("stg2", [M, 128], bf16)

    with tile.TileContext(nc) as tc:
        with (
            tc.tile_pool(name="persist", bufs=1) as pp,
            tc.tile_pool(name="psum", bufs=2, space="PSUM") as psp,
            tc.tile_pool(name="expo", bufs=2) as xp,
        ):
          for _rep in range(reps):
            # ---- load inputs, contiguous per partition ----
            # Layout [p, t, d] with m = p*MT + t (and n = p*CT + c): one 2KB
            # descriptor per partition, and the transposed column index below
            # comes out as exactly m (resp. n).
            samp_nat = pp.tile([128, CT, D], f32)
            nc.sync.dma_start(samp_nat[:], samples_d.ap().rearrange("(p c) d -> p c d", p=128))
            means_nat = pp.tile([128, MT, D], f32)
            stds_nat = pp.tile([128, MT, D], f32)
            nc.scalar.dma_start(stds_nat[:], stds_d.ap().rearrange("(p t) d -> p t d", p=128))
            nc.scalar.dma_start(means_nat[:], means_d.ap().rearrange("(p t) d -> p t d", p=128))

            # bf16 identity for PE-based transposes (sample side)
            identity = pp.tile([128, 128], bf16)
            make_identity(nc, identity[:])

            # ---- mean-side features, natural layout ----
            # DVE order is deliberate: the pass-1 (hi) features come first so
            # the pass-1 store/transpose chain can start while the lo features
            # are still being computed.
            r = pp.tile([128, MT, D], f32)       # 1/std
            mB = pp.tile([128, MT, D], f32)      # m * w' = -0.5*m/std
            t2 = pp.tile([128, MT, D], f32)      # m^2 * w'
            nc.vector.reciprocal(r[:], stds_nat[:])
            nc.vector.scalar_tensor_tensor(
                mB[:], means_nat[:], -0.5, r[:], op0=OP.mult, op1=OP.mult)

            packed1 = pp.tile([128, MT, 128], bf16)
            packed2 = pp.tile([128, MT, 128], bf16)
            # pass1 mean cols: [mB_hi, mB_hi, w'_hi, w'_hi]
            nc.vector.tensor_copy(packed1[:, :, 0:D], mB[:])                    # mB_hi
            nc.vector.tensor_scalar_mul(packed1[:, :, 2 * D:3 * D], r[:], -0.5)  # w'_hi
            nc.gpsimd.tensor_copy(packed1[:, :, D:2 * D], packed1[:, :, 0:D])
            nc.gpsimd.tensor_copy(packed1[:, :, 3 * D:4 * D], packed1[:, :, 2 * D:3 * D])
            # pass2 mean cols: [mB_lo, w'_lo, mq_hi, mq_lo]
            nc.vector.scalar_tensor_tensor(                                     # mB - mB_hi
                packed2[:, :, 0:D], mB[:], 1.0, packed1[:, :, 0:D],
                op0=OP.mult, op1=OP.subtract)
            nc.vector.scalar_tensor_tensor(                                     # -0.5r - w'_hi
                packed2[:, :, D:2 * D], r[:], -0.5, packed1[:, :, 2 * D:3 * D],
                op0=OP.mult, op1=OP.subtract)
            nc.vector.tensor_mul(t2[:], means_nat[:], mB[:])
            nc.vector.tensor_copy(packed2[:, :, 2 * D:3 * D], t2[:])            # mq_hi
            nc.vector.scalar_tensor_tensor(                                     # mq - mq_hi
                packed2[:, :, 3 * D:4 * D], t2[:], 1.0, packed2[:, :, 2 * D:3 * D],
                op0=OP.mult, op1=OP.subtract)

            # ---- sample-side features + PE transpose (no DRAM bounce) ----
            s2 = pp.tile([128, CT, D], f32)
            nc.vector.tensor_mul(s2[:], samp_nat[:], samp_nat[:])
            spacked = pp.tile([128, CT, 128], bf16)  # [sB_hi, sB_lo, s2_hi, s2_lo]
            nc.vector.tensor_scalar_mul(spacked[:, :, 0:D], samp_nat[:], -2.0)  # sB_hi
            nc.vector.scalar_tensor_tensor(
                spacked[:, :, D:2 * D], samp_nat[:], -2.0, spacked[:, :, 0:D],
                op0=OP.mult, op1=OP.subtract)                                   # sB_lo
            nc.vector.tensor_copy(spacked[:, :, 2 * D:3 * D], s2[:])            # s2_hi
            nc.vector.scalar_tensor_tensor(
                spacked[:, :, 3 * D:4 * D], s2[:], 1.0, spacked[:, :, 2 * D:3 * D],
                op0=OP.mult, op1=OP.subtract)                                   # s2_lo
            s1T = pp.tile([128, NSH], bf16)   # col n: [sB_hi, sB_lo, s2_hi, s2_lo]
            for c in range(CT):
                tp = psp.tile([128, 128], bf16, tag="ps")
                nc.tensor.transpose(tp[:], spacked[:, c, :], identity[:])
                nc.vector.tensor_copy(s1T[:, c * 128:(c + 1) * 128], tp[:])
            # pass2 sample rows: [sB_hi, s2_hi, ones(64)] (ones pair with mq rows)
            s2T = pp.tile([128, NSH], bf16)
            nc.gpsimd.memset(s2T[2 * D:4 * D, :], 1.0)
            nc.vector.tensor_copy(s2T[0:D, :], s1T[0:D, :])
            nc.vector.tensor_copy(s2T[D:2 * D, :], s1T[2 * D:3 * D, :])

            # ---- mean store + transpose in p-halves on the SP queue, pass1
            # chain first (ACT stays free for exp) ----
            rhs1 = pp.tile([128, M], bf16)
            rhs2 = pp.tile([128, M], bf16)
            stg1v = stg1_d.ap().rearrange("(p t) f -> p (t f)", p=128)
            stg2v = stg2_d.ap().rearrange("(p t) f -> p (t f)", p=128)
            for g in range(2):
                pg = slice(g * 64, (g + 1) * 64)
                rg = slice(g * 1024, (g + 1) * 1024)
                nc.sync.dma_start(stg1v[pg, :], packed1[pg, :, :])
                nc.sync.dma_start(rhs1[:, rg], stg1_d.ap()[rg, :], transpose=True)
            for g in range(2):
                pg = slice(g * 64, (g + 1) * 64)
                rg = slice(g * 1024, (g + 1) * 1024)
                nc.sync.dma_start(stg2v[pg, :], packed2[pg, :, :])
                nc.sync.dma_start(rhs2[:, rg], stg2_d.ap()[rg, :], transpose=True)

            # ---- main loop: 8 matmuls + one exp-accumulate per chunk ----
            ebias = pp.tile([128, 1], f32)   # exp bias: -ln(M) folds the 1/M mean
            nc.gpsimd.memset(ebias[:], -LN_M)
            dist_sb = pp.tile([128, CT], f32)
            for c in range(CT):
                ps = psp.tile([128, M], f32)  # 4 PSUM banks
                lhs1 = s1T[:, c * 128:(c + 1) * 128]
                lhs2 = s2T[:, c * 128:(c + 1) * 128]
                # all pass-1 slices share lhs1, then all pass-2 share lhs2:
                # one stationary load per pass instead of one per matmul
                for j in range(MJ):
                    sl = slice(j * 512, (j + 1) * 512)
                    nc.tensor.matmul(ps[:, sl], lhsT=lhs1, rhs=rhs1[:, sl],
                                     start=True, stop=False, skip_group_check=True)
                for j in range(MJ):
                    sl = slice(j * 512, (j + 1) * 512)
                    nc.tensor.matmul(ps[:, sl], lhsT=lhs2, rhs=rhs2[:, sl],
                                     start=False, stop=True, skip_group_check=True)
                eo = xp.tile([128, M], bf16)
                nc.scalar.activation(eo[:], ps[:], AF.Exp, bias=ebias[:],
                                     scale=1.0, accum_out=dist_sb[:, c:c + 1])

            # psum partition q of chunk c is n = q*CT + c, so the "(p c)" view
            # writes dist in natural order
            nc.sync.dma_start(dist_d.ap().rearrange("(p c) -> p c", p=128), dist_sb[:])

    nc.compile()
    return nc


def _get_nc():
    if "nc" not in _CACHE:
        _CACHE["nc"] = _build_nc()
    return _CACHE["nc"]


def kernel(samples: np.ndarray, means: np.ndarray, stds: np.ndarray) -> np.ndarray:
    from concourse.bass_utils import run_bass_kernel_spmd

    samples = np.ascontiguousarray(samples, dtype=np.float32)
    means = np.ascontiguousarray(means, dtype=np.float32)
    stds = np.ascontiguousarray(stds, dtype=np.float32)

    nc = _get_nc()
    in_maps = [
        {"samples": samples[i * NSH:(i + 1) * NSH], "means": means, "stds": stds}
        for i in range(N_CORES)
    ]
    res = run_bass_kernel_spmd(nc, in_maps, list(range(N_CORES)))
    dist = np.concatenate([res.results[i]["dist"] for i in range(N_CORES)])
    return (-dist + dist.max() + dist.min()).astype(np.float32)
